# revision 26
# baseline (speedup 1.0000x reference)
"""Trainium2 Bass kernel for 2-layer GAT (nn_GAT_3075196584311).

Strategy (8-core SPMD, 1D node partition by dst):
  - Table-based message passing: per layer a DRAM table holds, per node,
    [features fp16 (256) | alpha_src fp16 | alpha_dst fp16] in 768B rows.
    Each core computes rows for its own 6250 nodes (dense matmul on PE,
    fused alpha projections); tables are replicated via two chunked
    AllGathers per layer that overlap compute:
      * table order per core = [local id < 3200 nodes | rest], each group
        sorted by its alpha-phase in-degree. Chunk0 = rows 0:3200 (tfa),
        chunk1 = rows 3200:6250 (tfb).
      * edges split by SRC membership: src local id < 3200 -> beta edge
        (gathers tfa), else alpha edge (gathers tfb). So the beta phase
        only needs chunk0 (AllGather'd first) and runs while chunk1's
        AllGather is in flight.
  - Edges grouped by dst into 128-node blocks; each (block, k) gather tile
    holds the k-th incoming edge of 128 dst nodes; per-tile softmax
    weights from gathered alpha_src + per-dst alpha_dst (leaky relu on
    DVE, exp on ACT, no max-subtraction: |alpha| <~ 8). One batched DVE
    multiply weights all K slots; weight columns are appended to the
    moving tile so PSUM accumulates features AND softmax denominators in
    one matmul chain (identity lhsT). Beta partials staged to DRAM in
    beta order, gathered back and merged in PSUM during the alpha phase.
  - Layer 2 aggregates the 256-dim ELU features and applies W2 after
    aggregation (linearity), so both layers share the same table format
    and index tables. Gathers are spread over 4 SWDGE queues.
"""

import sys
import numpy as np

for _p in ("/opt/trn_rl_repo", "/opt/pypackages"):
    if _p not in sys.path:
        sys.path.insert(0, _p)

import concourse.bass as bass
import concourse.mybir as mybir
import concourse.tile as tile
from concourse import bacc
from concourse import bass_utils
from concourse.masks import make_identity

# problem constants
N = 50000
F_IN = 256
HID = 64
H = 4
OUT = 64
E = 800000
NEG = 0.2

NC = 8
NPC = N // NC            # 6250 nodes per core
P = 128
NBLK = (NPC + P - 1) // P  # 49
NSLOT = NBLK * P           # 6272
CH0 = 3200                 # chunk0 rows per core (25 blocks)
CH1 = NPC - CH0            # 3050
CBLK = CH0 // P            # 25 blocks in chunk0
ROWB = 384                 # u16 cols per table row (768 bytes)
KCH = 3                    # dense contraction chunks (384 rows)
OWN_CHUNK = 13             # blocks per B-own / aggB gather chunk

f16 = mybir.dt.float16
f32 = mybir.dt.float32
u16 = mybir.dt.uint16
i16 = mybir.dt.int16
Alu = mybir.AluOpType
Act = mybir.ActivationFunctionType

_CACHE = {}


# --------------------------------------------------------------------------
# host preprocessing
# --------------------------------------------------------------------------

def _wrap_idx(idx):
    """int array -> [128, ceil(n/16)] int16 wrapped layout for dma_gather."""
    n = len(idx)
    cols = (n + 15) // 16
    pad = np.zeros(cols * 16, np.int16)
    pad[:n] = idx.astype(np.int16)
    w = np.zeros((128, cols), np.int16)
    blk = pad.reshape(cols, 16).T
    for g in range(8):
        w[g * 16:(g + 1) * 16, :] = blk
    return w


def _preprocess(adj):
    src = np.concatenate([adj[0], np.arange(N)]).astype(np.int64)
    dst = np.concatenate([adj[1], np.arange(N)]).astype(np.int64)
    owner = dst // NPC
    is_beta = (src % NPC) < CH0     # edge phase by SRC local id

    srcs_by_core, lds_by_core = [], []
    acnt = np.zeros((NC, NPC), np.int64)
    bcnt = np.zeros((NC, NPC), np.int64)
    for c in range(NC):
        sel = owner == c
        s = src[sel]
        ld = dst[sel] - c * NPC
        srcs_by_core.append(s)
        lds_by_core.append(ld)
        sb = is_beta[sel]
        acnt[c] = np.bincount(ld[~sb], minlength=NPC)
        bcnt[c] = np.bincount(ld[sb], minlength=NPC)

    # table order: chunk0 ids then chunk1 ids, each sorted by alpha-degree
    permA, permB, rankA, rankB = [], [], [], []
    for c in range(NC):
        g0 = np.arange(CH0)
        g1 = np.arange(CH0, NPC)
        o0 = g0[np.argsort(-acnt[c][g0], kind="stable")]
        o1 = g1[np.argsort(-acnt[c][g1], kind="stable")]
        pa = np.concatenate([o0, o1])
        pb = np.argsort(-bcnt[c], kind="stable")
        permA.append(pa)
        permB.append(pb)
        rankA.append(np.argsort(pa, kind="stable"))
        rankB.append(np.argsort(pb, kind="stable"))

    # global (cross-core max) per-block tile counts
    KaG = np.zeros(NBLK, np.int64)
    KbG = np.zeros(NBLK, np.int64)
    for c in range(NC):
        a_s = acnt[c][permA[c]]
        b_s = bcnt[c][permB[c]]
        for i in range(NBLK):
            sl = slice(i * P, min((i + 1) * P, NPC))
            KaG[i] = max(KaG[i], a_s[sl].max())
            KbG[i] = max(KbG[i], b_s[sl].max())
    KaG = KaG.astype(int)
    KbG = KbG.astype(int)

    # gather row of node s: beta -> tfa row, alpha -> tfb row
    g_row = np.empty(N, np.int64)
    for c in range(NC):
        ra = rankA[c]
        ls = np.arange(NPC)
        rows = np.where(ls < CH0, c * CH0 + ra, c * CH1 + (ra - CH0))
        g_row[c * NPC:(c + 1) * NPC] = rows

    per_core = []
    for c in range(NC):
        s = srcs_by_core[c]
        ld = lds_by_core[c]
        rows = g_row[s]
        sb = (s % NPC) < CH0
        edgesA = [[] for _ in range(NPC)]
        edgesB = [[] for _ in range(NPC)]
        for e in range(len(s)):
            if sb[e]:
                edgesB[ld[e]].append(rows[e])
            else:
                edgesA[ld[e]].append(rows[e])

        def build(perm, edges, Ks):
            slots = int(P * sum(Ks))
            gidx = np.zeros(slots, np.int64)
            mask = np.full((P, sum(Ks)), -1e9, np.float32)
            off = 0
            t0 = 0
            for i in range(NBLK):
                K = Ks[i]
                for k in range(K):
                    for p in range(P):
                        r = i * P + p
                        node = perm[r] if r < NPC else -1
                        if node >= 0 and k < len(edges[node]):
                            gidx[off] = edges[node][k]
                            mask[p, t0 + k] = 0.0
                        off += 1
                t0 += K
            return gidx, mask

        gidxA, maskA = build(permA[c], edgesA, KaG)
        gidxB, maskB = build(permB[c], edgesB, KbG)

        # B-own rows (per B-rank, own-table row = a-rank of that node)
        bown = np.zeros(NSLOT, np.int64)
        bown[:NPC] = rankA[c][permB[c]]
        # aggB gather idx per a-rank: b-rank of that node
        aggb = np.zeros(NSLOT, np.int64)
        aggb[:NPC] = rankB[c][permA[c]]

        per_core.append(dict(
            gidxA=_wrap_idx(gidxA), maskA=maskA,
            gidxB=_wrap_idx(gidxB), maskB=maskB,
            bown=_wrap_idx(bown), aggb=_wrap_idx(aggb),
            permA=permA[c],
        ))

    return KaG, KbG, per_core


def _host_tensors(inputs, per_core):
    x = np.asarray(inputs["x"], np.float32)
    W1 = np.asarray(inputs["W1"], np.float32)
    as1 = np.asarray(inputs["att_src1"], np.float32)
    ad1 = np.asarray(inputs["att_dst1"], np.float32)
    b1 = np.asarray(inputs["b1"], np.float32)
    W2 = np.asarray(inputs["W2"], np.float32)
    as2 = np.asarray(inputs["att_src2"], np.float32)
    ad2 = np.asarray(inputs["att_dst2"], np.float32)
    b2 = np.asarray(inputs["b2"], np.float32)

    # dense rhs: [W1 | W1@Asrc | W1@Adst] with bias row; rows padded to 384
    A_src = np.zeros((H * HID, H), np.float32)
    A_dst = np.zeros((H * HID, H), np.float32)
    for h in range(H):
        A_src[h * HID:(h + 1) * HID, h] = as1[h]
        A_dst[h * HID:(h + 1) * HID, h] = ad1[h]
    wa1 = np.zeros((KCH * P, 264), np.float32)
    wa1[:F_IN, :256] = W1
    wa1[:F_IN, 256:260] = W1 @ A_src
    wa1[:F_IN, 260:264] = W1 @ A_dst
    wa1[F_IN, :256] = b1          # ones-row carries bias into h1'
    wa1_sb = wa1.reshape(KCH, P, 264).transpose(1, 0, 2).astype(np.float16)

    # layer-2 projections, replicated across partitions for row-space accum
    ws2 = W2 @ as2[0]             # [256]
    wd2 = W2 @ ad2[0]
    wsd2 = np.broadcast_to(
        np.stack([ws2, wd2], 0)[None], (P, 2, 256)).astype(np.float32)
    w2c = W2.reshape(2, P, OUT).transpose(1, 0, 2).astype(np.float32)
    b2r = b2.reshape(1, OUT).astype(np.float32)

    maps = []
    for c in range(NC):
        pc = per_core[c]
        xs = x[c * NPC:(c + 1) * NPC][pc["permA"]]       # table-order rows
        xT = np.zeros((KCH * P, NSLOT), np.float32)
        xT[:F_IN, :NPC] = xs.T
        xT[F_IN, :NPC] = 1.0                              # bias/ones row
        xT_sb = xT.reshape(KCH, P, NSLOT).transpose(1, 0, 2).astype(np.float16)
        maps.append(dict(
            xT=np.ascontiguousarray(xT_sb.reshape(P, KCH * NSLOT)),
            wa1=np.ascontiguousarray(wa1_sb.reshape(P, KCH * 264)),
            wsd2=np.ascontiguousarray(wsd2.reshape(P, 2 * 256)),
            w2c=np.ascontiguousarray(w2c.reshape(P, 2 * OUT)),
            b2r=b2r,
            gidxA=pc["gidxA"], maskA=pc["maskA"],
            gidxB=pc["gidxB"], maskB=pc["maskB"],
            bown=pc["bown"], aggb=pc["aggb"],
        ))
    return maps


# --------------------------------------------------------------------------
# device program
# --------------------------------------------------------------------------

def _build_program(KaG, KbG):
    TA, TB = int(sum(KaG)), int(sum(KbG))
    SA, SB = P * TA, P * TB

    nc = bacc.Bacc("TRN2", target_bir_lowering=False, debug=False,
                   num_devices=NC, num_swdge_queues=4)
    qctr = [0]

    def next_q():
        qctr[0] = (qctr[0] + 1) % 2
        return qctr[0]

    t_xT = nc.dram_tensor("xT", [P, KCH * NSLOT], f16, kind="ExternalInput")
    t_wa1 = nc.dram_tensor("wa1", [P, KCH * 264], f16, kind="ExternalInput")
    t_wsd2 = nc.dram_tensor("wsd2", [P, 2 * 256], f32, kind="ExternalInput")
    t_w2c = nc.dram_tensor("w2c", [P, 2 * OUT], f32, kind="ExternalInput")
    t_b2r = nc.dram_tensor("b2r", [1, OUT], f32, kind="ExternalInput")
    t_giA = nc.dram_tensor("gidxA", [P, SA // 16], i16, kind="ExternalInput")
    t_mkA = nc.dram_tensor("maskA", [P, TA], f32, kind="ExternalInput")
    t_giB = nc.dram_tensor("gidxB", [P, SB // 16], i16, kind="ExternalInput")
    t_mkB = nc.dram_tensor("maskB", [P, TB], f32, kind="ExternalInput")
    t_bown = nc.dram_tensor("bown", [P, NSLOT // 16], i16, kind="ExternalInput")
    t_aggb = nc.dram_tensor("aggb", [P, NSLOT // 16], i16, kind="ExternalInput")
    t_out = nc.dram_tensor("out", [NSLOT, OUT], f32, kind="ExternalOutput")

    with tile.TileContext(nc) as tc:
        with tc.tile_pool(name="const", bufs=1) as cp, \
             tc.tile_pool(name="dram", bufs=1, space="DRAM") as dp, \
             tc.tile_pool(name="psum_d", bufs=1, space="PSUM") as psd, \
             tc.tile_pool(name="psum_agg", bufs=4, space="PSUM") as psa, \
             tc.tile_pool(name="psum_tp", bufs=2, space="PSUM") as pst, \
             tc.tile_pool(name="psum_sm", bufs=1, space="PSUM") as pss, \
             tc.tile_pool(name="gat", bufs=3) as gp, \
             tc.tile_pool(name="own", bufs=2) as op_, \
             tc.tile_pool(name="wrk", bufs=4) as wp, \
             tc.tile_pool(name="stg", bufs=4) as sp:

            # ---- persistent tables / constants ----
            tab_own1 = dp.tile([NPC, ROWB], u16, name="tab_own1")
            tfa1 = dp.tile([NC * CH0, ROWB], u16, name="tfa1")
            tfb1 = dp.tile([NC * CH1, ROWB], u16, name="tfb1")
            tab_own2 = dp.tile([NPC, ROWB], u16, name="tab_own2")
            tfa2 = dp.tile([NC * CH0, ROWB], u16, name="tfa2")
            tfb2 = dp.tile([NC * CH1, ROWB], u16, name="tfb2")
            aggB1 = dp.tile([NSLOT, ROWB], u16, name="aggB1")
            aggB2 = dp.tile([NSLOT, ROWB], u16, name="aggB2")

            giA = cp.tile([P, SA // 16], i16)
            nc.sync.dma_start(giA[:], t_giA.ap())
            mkA = cp.tile([P, TA], f32)
            nc.sync.dma_start(mkA[:], t_mkA.ap())
            giB = cp.tile([P, SB // 16], i16)
            nc.sync.dma_start(giB[:], t_giB.ap())
            mkB = cp.tile([P, TB], f32)
            nc.sync.dma_start(mkB[:], t_mkB.ap())
            gbo = cp.tile([P, NSLOT // 16], i16)
            nc.sync.dma_start(gbo[:], t_bown.ap())
            gab = cp.tile([P, NSLOT // 16], i16)
            nc.sync.dma_start(gab[:], t_aggb.ap())
            wsdrep = cp.tile([P, 2, 256], f32)
            nc.sync.dma_start(wsdrep[:], t_wsd2.ap())
            w2c = cp.tile([P, 2, OUT], f32)
            nc.sync.dma_start(w2c[:], t_w2c.ap())
            b2r = cp.tile([1, OUT], f32)
            nc.sync.dma_start(b2r[:], t_b2r.ap())

            id16 = cp.tile([P, P], f16)
            make_identity(nc, id16[:])
            id32 = cp.tile([P, P], f32)
            make_identity(nc, id32[:])
            ones1 = cp.tile([1, P], f32)
            nc.vector.memset(ones1[:], 1.0)

            adst1 = cp.tile([P, NBLK, H], f32)
            adst2 = cp.tile([P, NBLK], f32)
            adstB1 = cp.tile([P, NBLK, H], f16)
            adstB2 = cp.tile([P, NBLK], f16)

            # ---- dense phase: h1' rows + alphas (xT streamed per block) ----
            wa1 = cp.tile([P, KCH, 264], f16)
            nc.sync.dma_start(wa1[:], t_wa1.ap())

            xT_ap = t_xT.ap().rearrange("p (k s) -> p k s", k=KCH)
            for r in range(NBLK):
                xb = sp.tile([P, KCH, P], f16, tag="xb")
                nc.sync.dma_start(xb[:], xT_ap[:, :, r * P:(r + 1) * P])
                ps = psd.tile([P, 264], f32, space="PSUM")
                for kc in range(KCH):
                    nc.tensor.matmul(ps[:], xb[:, kc, :],
                                     wa1[:, kc, :],
                                     start=(kc == 0), stop=(kc == KCH - 1))
                stg = sp.tile([P, ROWB], u16, tag="stage")
                nc.vector.tensor_copy(out=stg[:, 0:264].bitcast(f16), in_=ps[:])
                nc.vector.tensor_copy(out=adst1[:, r, :], in_=ps[:, 260:264])
                rows = min(NPC - r * P, P)
                nc.sync.dma_start(tab_own1[r * P:r * P + rows, :],
                                  stg[0:rows, :])
                if r == CBLK - 1:
                    nc.gpsimd.collective_compute(
                        "AllGather", Alu.bypass,
                        replica_groups=[list(range(NC))],
                        ins=[tab_own1[0:CH0, :].opt()],
                        outs=[tfa1.opt()])

            nc.gpsimd.collective_compute(
                "AllGather", Alu.bypass,
                replica_groups=[list(range(NC))],
                ins=[tab_own1[CH0:NPC, :].opt()],
                outs=[tfb1.opt()])

            # ---- B-own alpha_dst gathers (from own shard) ----
            def load_adstB(tab_own, layer):
                for c0 in range(0, NBLK, OWN_CHUNK):
                    nb = min(OWN_CHUNK, NBLK - c0)
                    g = op_.tile([P, OWN_CHUNK, ROWB], u16, tag="bown")
                    nc.gpsimd.dma_gather(
                        out_ap=g[:, 0:nb, :], in_ap=tab_own[:],
                        idxs_ap=gbo[:, c0 * 8:(c0 + nb) * 8],
                        num_idxs=nb * P, num_idxs_reg=nb * P,
                        elem_size=ROWB, single_packet=False,
                        queue_num=next_q())
                    if layer == 1:
                        nc.vector.tensor_copy(
                            out=adstB1[:, c0:c0 + nb, :],
                            in_=g[:, 0:nb, 260:264].bitcast(f16))
                    else:
                        nc.vector.tensor_copy(
                            out=adstB2[:, c0:c0 + nb],
                            in_=g[:, 0:nb, 257].bitcast(f16))

            # ---- edge aggregation pass (shared for both phases/layers) ----
            def agg_block(K, t0, gi, mk, tab_src, nh, adst_ap_fn,
                          stop_last=True):
                """returns psum ps[P, 256+nh]: cols 0:256 weighted feature sum,
                cols 256:256+nh softmax denominators. With stop_last=False the
                PSUM group stays open for the caller's merge matmul."""
                g = gp.tile([P, K, ROWB], u16, tag="gtile")
                nc.gpsimd.dma_gather(
                    out_ap=g[:], in_ap=tab_src,
                    idxs_ap=gi[:, t0 * 8:(t0 + K) * 8],
                    num_idxs=K * P, num_idxs_reg=K * P, elem_size=ROWB,
                    single_packet=False, queue_num=next_q())
                t = wp.tile([P, nh, K], f32, tag="t")
                for h in range(nh):
                    nc.vector.scalar_tensor_tensor(
                        out=t[:, h, :],
                        in0=g[:, :, 256 + h].bitcast(f16),
                        scalar=adst_ap_fn(h),
                        in1=mk[:, t0:t0 + K],
                        op0=Alu.add, op1=Alu.add)
                nc.vector.scalar_tensor_tensor(
                    out=t[:], in0=t[:], scalar=NEG, in1=t[:],
                    op0=Alu.mult, op1=Alu.max)
                # tmp holds, per edge slot k: [w*g (256) | w per head (nh)]
                tmp = wp.tile([P, K, 256 + nh], f16, tag="tmp")
                if nh == 1:
                    wt32 = wp.tile([P, K], f32, tag="wt32")
                    nc.scalar.activation(wt32[:, None, :], t[:], Act.Exp)
                    nc.vector.tensor_copy(
                        out=tmp[:, :, 256].bitcast(f16)[:, :, None],
                        in_=wt32[:, :, None])
                else:
                    nc.scalar.activation(
                        tmp[:, :, 256:256 + nh].rearrange("p k h -> p h k"),
                        t[:], Act.Exp)
                if nh == 1:
                    half = K // 2
                    for k in range(half):
                        nc.scalar.mul(tmp[:, k, 0:256],
                                      g[:, k, 0:256].bitcast(f16),
                                      wt32[:, k:k + 1])
                    if half < K:
                        nc.vector.tensor_tensor(
                            out=tmp[:, half:K, 0:256],
                            in0=g[:, half:K, 0:256].bitcast(f16),
                            in1=tmp[:, half:K, 256:257].to_broadcast(
                                [P, K - half, 256]),
                            op=Alu.mult)
                else:
                    nc.vector.tensor_tensor(
                        out=tmp[:, :, 0:256].rearrange(
                            "p k (h c) -> p k h c", h=nh),
                        in0=g[:, :, 0:256].bitcast(f16)
                             .rearrange("p k (h c) -> p k h c", h=nh),
                        in1=tmp[:, :, 256:256 + nh][:, :, :, None].to_broadcast(
                            [P, K, nh, 256 // nh]),
                        op=Alu.mult)
                ps = psa.tile([P, 256 + nh], f32, space="PSUM", tag="agg")
                for k in range(K):
                    nc.tensor.matmul(ps[:], id16[:], tmp[:, k, :],
                                     start=(k == 0),
                                     stop=(stop_last and k == K - 1))
                return ps

            def b_phase(tfa, aggB, layer):
                nh = H if layer == 1 else 1
                t0 = 0
                for j in range(NBLK):
                    K = KbG[j]
                    if layer == 1:
                        fn = lambda h, j=j: adstB1[:, j, h:h + 1]
                    else:
                        fn = lambda h, j=j: adstB2[:, j:j + 1]
                    ps = agg_block(K, t0, giB, mkB, tfa[:], nh, fn)
                    stg = sp.tile([P, ROWB], u16, tag="stage")
                    nc.vector.tensor_copy(
                        out=stg[:, 0:256 + nh].bitcast(f16),
                        in_=ps[:])
                    nc.sync.dma_start(aggB[j * P:(j + 1) * P, :], stg[:])
                    t0 += K

            def a_phase(tfb, aggB, layer):
                nh = H if layer == 1 else 1
                t0 = 0
                for i in range(NBLK):
                    K = KaG[i]
                    if layer == 1:
                        fn = lambda h, i=i: adst1[:, i, h:h + 1]
                    else:
                        fn = lambda h, i=i: adst2[:, i:i + 1]
                    ps = agg_block(K, t0, giA, mkA, tfb[:], nh, fn,
                                   stop_last=False)
                    # merge gathered B aggregate (features + denoms) in PSUM
                    c0 = (i // OWN_CHUNK) * OWN_CHUNK
                    if i % OWN_CHUNK == 0:
                        nb = min(OWN_CHUNK, NBLK - c0)
                        gb = op_.tile([P, OWN_CHUNK, ROWB], u16, tag="aggbg")
                        nc.gpsimd.dma_gather(
                            out_ap=gb[:, 0:nb, :], in_ap=aggB[:],
                            idxs_ap=gab[:, c0 * 8:(c0 + nb) * 8],
                            num_idxs=nb * P, num_idxs_reg=nb * P,
                            elem_size=ROWB, single_packet=False,
                            queue_num=next_q())
                        a_phase.gb = gb
                    gb = a_phase.gb
                    jj = i - c0
                    nc.tensor.matmul(
                        ps[:], id16[:],
                        gb[:, jj, 0:256 + nh].bitcast(f16),
                        start=False, stop=True)
                    rec = wp.tile([P, nh], f32, tag="rec")
                    nc.vector.reciprocal(rec[:], ps[:, 256:256 + nh])
                    xv = wp.tile([P, 256], f32, tag="xdiv")
                    nc.vector.tensor_tensor(
                        out=xv[:].rearrange("p (h c) -> p h c", h=nh),
                        in0=ps[:, 0:256].rearrange("p (h c) -> p h c", h=nh),
                        in1=rec[:, :, None].to_broadcast([P, nh, 256 // nh]),
                        op=Alu.mult)
                    if layer == 1:
                        epilogue1(i, xv)
                    else:
                        epilogue2(i, xv)
                    t0 += K

            def epilogue1(i, xv):
                # z = elu(xv); stage [z f16 256 | asrc2 | adst2]
                u = wp.tile([P, 256], f32, tag="eluu")
                nc.vector.tensor_scalar_min(out=u[:], in0=xv[:], scalar1=0.0)
                e = wp.tile([P, 256], f32, tag="elue")
                nc.scalar.activation(e[:], u[:], Act.Exp)
                stg = sp.tile([P, ROWB], u16, tag="stage")
                z16 = stg[:, 0:256].bitcast(f16)
                nc.vector.scalar_tensor_tensor(
                    out=z16, in0=e[:], scalar=-1.0, in1=xv[:],
                    op0=Alu.add, op1=Alu.max)
                # alpha2 = z @ [ws2|wd2] via fused mult+row-accumulate
                pa = wp.tile([P, 2], f32, tag="pa")
                for cch in range(2):
                    scr = wp.tile([P, 256], f32, tag="a2scr")
                    nc.vector.scalar_tensor_tensor(
                        out=scr[:], in0=z16, scalar=1.0,
                        in1=wsdrep[:, cch, :],
                        op0=Alu.mult, op1=Alu.mult,
                        accum_out=pa[:, cch:cch + 1])
                nc.vector.tensor_copy(out=stg[:, 256:258].bitcast(f16),
                                      in_=pa[:])
                nc.vector.tensor_copy(out=adst2[:, i:i + 1], in_=pa[:, 1:2])
                rows = min(NPC - i * P, P)
                if rows > 0:
                    nc.sync.dma_start(tab_own2[i * P:i * P + rows, :],
                                      stg[0:rows, :])
                if i == CBLK - 1:
                    nc.gpsimd.collective_compute(
                        "AllGather", Alu.bypass,
                        replica_groups=[list(range(NC))],
                        ins=[tab_own2[0:CH0, :].opt()],
                        outs=[tfa2.opt()])

            def epilogue2(i, xv):
                po = pss.tile([P, OUT], f32, space="PSUM", tag="out2")
                for cch in range(2):
                    pt = pst.tile([P, P], f32, space="PSUM", tag="tpose")
                    nc.tensor.transpose(pt[:], xv[:, cch * P:(cch + 1) * P],
                                        id32[:])
                    xt = wp.tile([P, P], f32, tag="xt")
                    nc.vector.tensor_copy(out=xt[:], in_=pt[:])
                    nc.tensor.matmul(po[:], xt[:], w2c[:, cch, :],
                                     start=(cch == 0), stop=False)
                nc.tensor.matmul(po[:], ones1[:], b2r[:],
                                 start=False, stop=True)
                # log_softmax over 64 cols
                m = wp.tile([P, 1], f32, tag="lsm")
                nc.vector.reduce_max(m[:], po[:], axis=mybir.AxisListType.X)
                sft = wp.tile([P, OUT], f32, tag="lss")
                nc.vector.tensor_scalar_sub(out=sft[:], in0=po[:], scalar1=m[:])
                ex = wp.tile([P, OUT], f32, tag="lse")
                sm = wp.tile([P, 1], f32, tag="lsum")
                nc.scalar.activation(ex[:], sft[:], Act.Exp, accum_out=sm[:])
                ls = wp.tile([P, 1], f32, tag="lls")
                nc.scalar.activation(ls[:], sm[:], Act.Ln)
                res = wp.tile([P, OUT], f32, tag="lres")
                nc.vector.tensor_scalar_sub(out=res[:], in0=sft[:], scalar1=ls[:])
                nc.sync.dma_start(t_out.ap()[i * P:(i + 1) * P, :], res[:])

            # ---- layer 1 ----
            load_adstB(tab_own1, 1)
            b_phase(tfa1, aggB1, 1)
            a_phase(tfb1, aggB1, 1)

            # ---- finish layer-2 table AllGather (chunk1) ----
            nc.gpsimd.collective_compute(
                "AllGather", Alu.bypass,
                replica_groups=[list(range(NC))],
                ins=[tab_own2[CH0:NPC, :].opt()],
                outs=[tfb2.opt()])

            # ---- layer 2 ----
            load_adstB(tab_own2, 2)
            b_phase(tfa2, aggB2, 2)
            a_phase(tfb2, aggB2, 2)

    nc.compile()
    return nc


# --------------------------------------------------------------------------
# entry point
# --------------------------------------------------------------------------

def kernel(**inputs):
    adj = np.asarray(inputs["adj"]).astype(np.int64)
    key = adj.tobytes()[:64] + adj.tobytes()[-64:]
    if "plan" not in _CACHE or _CACHE.get("key") != key:
        KaG, KbG, per_core = _preprocess(adj)
        nc = _build_program(KaG, KbG)
        _CACHE.update(plan=(KaG, KbG, per_core), nc=nc, key=key)
    KaG, KbG, per_core = _CACHE["plan"]
    nc = _CACHE["nc"]

    maps = _host_tensors(inputs, per_core)
    res = bass_utils.run_bass_kernel_spmd(nc, maps, core_ids=list(range(NC)))

    out = np.empty((N, OUT), np.float32)
    for c in range(NC):
        o = res.results[c]["out"][:NPC]
        out[c * NPC + per_core[c]["permA"]] = o
    return out


# revision 27
# speedup vs baseline: 1.0012x; 1.0012x over previous
"""Trainium2 Bass kernel for 2-layer GAT (nn_GAT_3075196584311).

Strategy (8-core SPMD, 1D node partition by dst):
  - Table-based message passing: per layer a DRAM table holds, per node,
    [features fp16 (256) | alpha_src fp16 | alpha_dst fp16] in 768B rows.
    Each core computes rows for its own 6250 nodes (dense matmul on PE,
    fused alpha projections); tables are replicated via two chunked
    AllGathers per layer that overlap compute:
      * table order per core = [local id < 3200 nodes | rest], each group
        sorted by its alpha-phase in-degree. Chunk0 = rows 0:3200 (tfa),
        chunk1 = rows 3200:6250 (tfb).
      * edges split by SRC membership: src local id < 3200 -> beta edge
        (gathers tfa), else alpha edge (gathers tfb). So the beta phase
        only needs chunk0 (AllGather'd first) and runs while chunk1's
        AllGather is in flight.
  - Edges grouped by dst into 128-node blocks; each (block, k) gather tile
    holds the k-th incoming edge of 128 dst nodes; per-tile softmax
    weights from gathered alpha_src + per-dst alpha_dst (leaky relu on
    DVE, exp on ACT, no max-subtraction: |alpha| <~ 8). One batched DVE
    multiply weights all K slots; weight columns are appended to the
    moving tile so PSUM accumulates features AND softmax denominators in
    one matmul chain (identity lhsT). Beta partials staged to DRAM in
    beta order, gathered back and merged in PSUM during the alpha phase.
  - Layer 2 aggregates the 256-dim ELU features and applies W2 after
    aggregation (linearity), so both layers share the same table format
    and index tables. Gathers are spread over 4 SWDGE queues.
"""

import sys
import numpy as np

for _p in ("/opt/trn_rl_repo", "/opt/pypackages"):
    if _p not in sys.path:
        sys.path.insert(0, _p)

import concourse.bass as bass
import concourse.mybir as mybir
import concourse.tile as tile
from concourse import bacc
from concourse import bass_utils
from concourse.masks import make_identity

# problem constants
N = 50000
F_IN = 256
HID = 64
H = 4
OUT = 64
E = 800000
NEG = 0.2

NC = 8
NPC = N // NC            # 6250 nodes per core
P = 128
NBLK = (NPC + P - 1) // P  # 49
NSLOT = NBLK * P           # 6272
CH0 = 3200                 # chunk0 rows per core (25 blocks)
CH1 = NPC - CH0            # 3050
CBLK = CH0 // P            # 25 blocks in chunk0
ROWB = 384                 # u16 cols per table row (768 bytes)
KCH = 3                    # dense contraction chunks (384 rows)
OWN_CHUNK = 13             # blocks per B-own / aggB gather chunk

f16 = mybir.dt.float16
f32 = mybir.dt.float32
u16 = mybir.dt.uint16
i16 = mybir.dt.int16
Alu = mybir.AluOpType
Act = mybir.ActivationFunctionType

_CACHE = {}


# --------------------------------------------------------------------------
# host preprocessing
# --------------------------------------------------------------------------

def _wrap_idx(idx):
    """int array -> [128, ceil(n/16)] int16 wrapped layout for dma_gather."""
    n = len(idx)
    cols = (n + 15) // 16
    pad = np.zeros(cols * 16, np.int16)
    pad[:n] = idx.astype(np.int16)
    w = np.zeros((128, cols), np.int16)
    blk = pad.reshape(cols, 16).T
    for g in range(8):
        w[g * 16:(g + 1) * 16, :] = blk
    return w


def _preprocess(adj):
    src = np.concatenate([adj[0], np.arange(N)]).astype(np.int64)
    dst = np.concatenate([adj[1], np.arange(N)]).astype(np.int64)
    owner = dst // NPC
    is_beta = (src % NPC) < CH0     # edge phase by SRC local id

    srcs_by_core, lds_by_core = [], []
    acnt = np.zeros((NC, NPC), np.int64)
    bcnt = np.zeros((NC, NPC), np.int64)
    for c in range(NC):
        sel = owner == c
        s = src[sel]
        ld = dst[sel] - c * NPC
        srcs_by_core.append(s)
        lds_by_core.append(ld)
        sb = is_beta[sel]
        acnt[c] = np.bincount(ld[~sb], minlength=NPC)
        bcnt[c] = np.bincount(ld[sb], minlength=NPC)

    # table order: chunk0 ids then chunk1 ids, each sorted by alpha-degree
    permA, permB, rankA, rankB = [], [], [], []
    for c in range(NC):
        g0 = np.arange(CH0)
        g1 = np.arange(CH0, NPC)
        o0 = g0[np.argsort(-acnt[c][g0], kind="stable")]
        o1 = g1[np.argsort(-acnt[c][g1], kind="stable")]
        pa = np.concatenate([o0, o1])
        pb = np.argsort(-bcnt[c], kind="stable")
        permA.append(pa)
        permB.append(pb)
        rankA.append(np.argsort(pa, kind="stable"))
        rankB.append(np.argsort(pb, kind="stable"))

    # global (cross-core max) per-block tile counts
    KaG = np.zeros(NBLK, np.int64)
    KbG = np.zeros(NBLK, np.int64)
    for c in range(NC):
        a_s = acnt[c][permA[c]]
        b_s = bcnt[c][permB[c]]
        for i in range(NBLK):
            sl = slice(i * P, min((i + 1) * P, NPC))
            KaG[i] = max(KaG[i], a_s[sl].max())
            KbG[i] = max(KbG[i], b_s[sl].max())
    KaG = KaG.astype(int)
    KbG = KbG.astype(int)

    # gather row of node s: beta -> tfa row, alpha -> tfb row
    g_row = np.empty(N, np.int64)
    for c in range(NC):
        ra = rankA[c]
        ls = np.arange(NPC)
        rows = np.where(ls < CH0, c * CH0 + ra, c * CH1 + (ra - CH0))
        g_row[c * NPC:(c + 1) * NPC] = rows

    per_core = []
    for c in range(NC):
        s = srcs_by_core[c]
        ld = lds_by_core[c]
        rows = g_row[s]
        sb = (s % NPC) < CH0
        edgesA = [[] for _ in range(NPC)]
        edgesB = [[] for _ in range(NPC)]
        for e in range(len(s)):
            if sb[e]:
                edgesB[ld[e]].append(rows[e])
            else:
                edgesA[ld[e]].append(rows[e])

        def build(perm, edges, Ks):
            slots = int(P * sum(Ks))
            gidx = np.zeros(slots, np.int64)
            mask = np.full((P, sum(Ks)), -1e9, np.float32)
            off = 0
            t0 = 0
            for i in range(NBLK):
                K = Ks[i]
                for k in range(K):
                    for p in range(P):
                        r = i * P + p
                        node = perm[r] if r < NPC else -1
                        if node >= 0 and k < len(edges[node]):
                            gidx[off] = edges[node][k]
                            mask[p, t0 + k] = 0.0
                        off += 1
                t0 += K
            return gidx, mask

        gidxA, maskA = build(permA[c], edgesA, KaG)
        gidxB, maskB = build(permB[c], edgesB, KbG)

        # B-own rows (per B-rank, own-table row = a-rank of that node)
        bown = np.zeros(NSLOT, np.int64)
        bown[:NPC] = rankA[c][permB[c]]
        # aggB gather idx per a-rank: b-rank of that node
        aggb = np.zeros(NSLOT, np.int64)
        aggb[:NPC] = rankB[c][permA[c]]

        per_core.append(dict(
            gidxA=_wrap_idx(gidxA), maskA=maskA,
            gidxB=_wrap_idx(gidxB), maskB=maskB,
            bown=_wrap_idx(bown), aggb=_wrap_idx(aggb),
            permA=permA[c],
        ))

    return KaG, KbG, per_core


def _host_tensors(inputs, per_core):
    x = np.asarray(inputs["x"], np.float32)
    W1 = np.asarray(inputs["W1"], np.float32)
    as1 = np.asarray(inputs["att_src1"], np.float32)
    ad1 = np.asarray(inputs["att_dst1"], np.float32)
    b1 = np.asarray(inputs["b1"], np.float32)
    W2 = np.asarray(inputs["W2"], np.float32)
    as2 = np.asarray(inputs["att_src2"], np.float32)
    ad2 = np.asarray(inputs["att_dst2"], np.float32)
    b2 = np.asarray(inputs["b2"], np.float32)

    # dense rhs: [W1 | W1@Asrc | W1@Adst] with bias row; rows padded to 384
    A_src = np.zeros((H * HID, H), np.float32)
    A_dst = np.zeros((H * HID, H), np.float32)
    for h in range(H):
        A_src[h * HID:(h + 1) * HID, h] = as1[h]
        A_dst[h * HID:(h + 1) * HID, h] = ad1[h]
    wa1 = np.zeros((KCH * P, 264), np.float32)
    wa1[:F_IN, :256] = W1
    wa1[:F_IN, 256:260] = W1 @ A_src
    wa1[:F_IN, 260:264] = W1 @ A_dst
    wa1[F_IN, :256] = b1          # ones-row carries bias into h1'
    wa1_sb = wa1.reshape(KCH, P, 264).transpose(1, 0, 2).astype(np.float16)

    # layer-2 projections, replicated across partitions for row-space accum
    ws2 = W2 @ as2[0]             # [256]
    wd2 = W2 @ ad2[0]
    wsd2 = np.broadcast_to(
        np.stack([ws2, wd2], 0)[None], (P, 2, 256)).astype(np.float32)
    w2c = W2.reshape(2, P, OUT).transpose(1, 0, 2).astype(np.float32)
    b2r = b2.reshape(1, OUT).astype(np.float32)

    maps = []
    for c in range(NC):
        pc = per_core[c]
        xs = x[c * NPC:(c + 1) * NPC][pc["permA"]]       # table-order rows
        xT = np.zeros((KCH * P, NSLOT), np.float32)
        xT[:F_IN, :NPC] = xs.T
        xT[F_IN, :NPC] = 1.0                              # bias/ones row
        xT_sb = xT.reshape(KCH, P, NSLOT).transpose(1, 0, 2).astype(np.float16)
        maps.append(dict(
            xT=np.ascontiguousarray(xT_sb.reshape(P, KCH * NSLOT)),
            wa1=np.ascontiguousarray(wa1_sb.reshape(P, KCH * 264)),
            wsd2=np.ascontiguousarray(wsd2.reshape(P, 2 * 256)),
            w2c=np.ascontiguousarray(w2c.reshape(P, 2 * OUT)),
            b2r=b2r,
            gidxA=pc["gidxA"], maskA=pc["maskA"],
            gidxB=pc["gidxB"], maskB=pc["maskB"],
            bown=pc["bown"], aggb=pc["aggb"],
        ))
    return maps


# --------------------------------------------------------------------------
# device program
# --------------------------------------------------------------------------

def _build_program(KaG, KbG):
    TA, TB = int(sum(KaG)), int(sum(KbG))
    SA, SB = P * TA, P * TB

    nc = bacc.Bacc("TRN2", target_bir_lowering=False, debug=False,
                   num_devices=NC, num_swdge_queues=4)
    qctr = [0]

    def next_q():
        qctr[0] = (qctr[0] + 1) % 2
        return qctr[0]

    t_xT = nc.dram_tensor("xT", [P, KCH * NSLOT], f16, kind="ExternalInput")
    t_wa1 = nc.dram_tensor("wa1", [P, KCH * 264], f16, kind="ExternalInput")
    t_wsd2 = nc.dram_tensor("wsd2", [P, 2 * 256], f32, kind="ExternalInput")
    t_w2c = nc.dram_tensor("w2c", [P, 2 * OUT], f32, kind="ExternalInput")
    t_b2r = nc.dram_tensor("b2r", [1, OUT], f32, kind="ExternalInput")
    t_giA = nc.dram_tensor("gidxA", [P, SA // 16], i16, kind="ExternalInput")
    t_mkA = nc.dram_tensor("maskA", [P, TA], f32, kind="ExternalInput")
    t_giB = nc.dram_tensor("gidxB", [P, SB // 16], i16, kind="ExternalInput")
    t_mkB = nc.dram_tensor("maskB", [P, TB], f32, kind="ExternalInput")
    t_bown = nc.dram_tensor("bown", [P, NSLOT // 16], i16, kind="ExternalInput")
    t_aggb = nc.dram_tensor("aggb", [P, NSLOT // 16], i16, kind="ExternalInput")
    t_out = nc.dram_tensor("out", [NSLOT, OUT], f32, kind="ExternalOutput")

    with tile.TileContext(nc) as tc:
        with tc.tile_pool(name="const", bufs=1) as cp, \
             tc.tile_pool(name="dram", bufs=1, space="DRAM") as dp, \
             tc.tile_pool(name="psum_d", bufs=1, space="PSUM") as psd, \
             tc.tile_pool(name="psum_agg", bufs=4, space="PSUM") as psa, \
             tc.tile_pool(name="psum_tp", bufs=2, space="PSUM") as pst, \
             tc.tile_pool(name="psum_sm", bufs=1, space="PSUM") as pss, \
             tc.tile_pool(name="gat", bufs=3) as gp, \
             tc.tile_pool(name="own", bufs=2) as op_, \
             tc.tile_pool(name="wrk", bufs=4) as wp, \
             tc.tile_pool(name="stg", bufs=4) as sp:

            # ---- persistent tables / constants ----
            tab_own1 = dp.tile([NPC, ROWB], u16, name="tab_own1")
            tfa1 = dp.tile([NC * CH0, ROWB], u16, name="tfa1")
            tfb1 = dp.tile([NC * CH1, ROWB], u16, name="tfb1")
            tab_own2 = dp.tile([NPC, ROWB], u16, name="tab_own2")
            tfa2 = dp.tile([NC * CH0, ROWB], u16, name="tfa2")
            tfb2 = dp.tile([NC * CH1, ROWB], u16, name="tfb2")
            aggB1 = dp.tile([NSLOT, ROWB], u16, name="aggB1")
            aggB2 = dp.tile([NSLOT, ROWB], u16, name="aggB2")

            giA = cp.tile([P, SA // 16], i16)
            nc.sync.dma_start(giA[:], t_giA.ap())
            mkA = cp.tile([P, TA], f32)
            nc.sync.dma_start(mkA[:], t_mkA.ap())
            giB = cp.tile([P, SB // 16], i16)
            nc.sync.dma_start(giB[:], t_giB.ap())
            mkB = cp.tile([P, TB], f32)
            nc.sync.dma_start(mkB[:], t_mkB.ap())
            gbo = cp.tile([P, NSLOT // 16], i16)
            nc.sync.dma_start(gbo[:], t_bown.ap())
            gab = cp.tile([P, NSLOT // 16], i16)
            nc.sync.dma_start(gab[:], t_aggb.ap())
            wsdrep = cp.tile([P, 2, 256], f32)
            nc.sync.dma_start(wsdrep[:], t_wsd2.ap())
            w2c = cp.tile([P, 2, OUT], f32)
            nc.sync.dma_start(w2c[:], t_w2c.ap())
            b2r = cp.tile([1, OUT], f32)
            nc.sync.dma_start(b2r[:], t_b2r.ap())

            id16 = cp.tile([P, P], f16)
            make_identity(nc, id16[:])
            id32 = cp.tile([P, P], f32)
            make_identity(nc, id32[:])
            ones1 = cp.tile([1, P], f32)
            nc.vector.memset(ones1[:], 1.0)

            adst1 = cp.tile([P, NBLK, H], f32)
            adst2 = cp.tile([P, NBLK], f32)
            adstB1 = cp.tile([P, NBLK, H], f16)
            adstB2 = cp.tile([P, NBLK], f16)

            # ---- dense phase: h1' rows + alphas (xT streamed per block) ----
            wa1 = cp.tile([P, KCH, 264], f16)
            nc.sync.dma_start(wa1[:], t_wa1.ap())

            xT_ap = t_xT.ap().rearrange("p (k s) -> p k s", k=KCH)
            for r in range(NBLK):
                xb = sp.tile([P, KCH, P], f16, tag="xb")
                nc.sync.dma_start(xb[:], xT_ap[:, :, r * P:(r + 1) * P])
                ps = psd.tile([P, 264], f32, space="PSUM")
                for kc in range(KCH):
                    nc.tensor.matmul(ps[:], xb[:, kc, :],
                                     wa1[:, kc, :],
                                     start=(kc == 0), stop=(kc == KCH - 1))
                stg = sp.tile([P, ROWB], u16, tag="stage")
                nc.vector.tensor_copy(out=stg[:, 0:264].bitcast(f16), in_=ps[:])
                nc.vector.tensor_copy(out=adst1[:, r, :], in_=ps[:, 260:264])
                rows = min(NPC - r * P, P)
                nc.sync.dma_start(tab_own1[r * P:r * P + rows, :],
                                  stg[0:rows, :])
                if r == CBLK - 1:
                    nc.gpsimd.collective_compute(
                        "AllGather", Alu.bypass,
                        replica_groups=[list(range(NC))],
                        ins=[tab_own1[0:CH0, :].opt()],
                        outs=[tfa1.opt()])

            nc.gpsimd.collective_compute(
                "AllGather", Alu.bypass,
                replica_groups=[list(range(NC))],
                ins=[tab_own1[CH0:NPC, :].opt()],
                outs=[tfb1.opt()])

            # ---- B-own alpha_dst gathers (from own shard) ----
            def load_adstB(tab_own, layer):
                for c0 in range(0, NBLK, OWN_CHUNK):
                    nb = min(OWN_CHUNK, NBLK - c0)
                    g = op_.tile([P, OWN_CHUNK, ROWB], u16, tag="bown")
                    nc.gpsimd.dma_gather(
                        out_ap=g[:, 0:nb, :], in_ap=tab_own[:],
                        idxs_ap=gbo[:, c0 * 8:(c0 + nb) * 8],
                        num_idxs=nb * P, num_idxs_reg=nb * P,
                        elem_size=ROWB, single_packet=False,
                        queue_num=next_q())
                    if layer == 1:
                        nc.vector.tensor_copy(
                            out=adstB1[:, c0:c0 + nb, :],
                            in_=g[:, 0:nb, 260:264].bitcast(f16))
                    else:
                        nc.vector.tensor_copy(
                            out=adstB2[:, c0:c0 + nb],
                            in_=g[:, 0:nb, 257].bitcast(f16))

            # ---- edge aggregation pass (shared for both phases/layers) ----
            def agg_block(K, t0, gi, mk, tab_src, nh, adst_ap_fn,
                          stop_last=True):
                """returns psum ps[P, 256+nh]: cols 0:256 weighted feature sum,
                cols 256:256+nh softmax denominators. With stop_last=False the
                PSUM group stays open for the caller's merge matmul."""
                g = gp.tile([P, K, ROWB], u16, tag="gtile")
                nc.gpsimd.dma_gather(
                    out_ap=g[:], in_ap=tab_src,
                    idxs_ap=gi[:, t0 * 8:(t0 + K) * 8],
                    num_idxs=K * P, num_idxs_reg=K * P, elem_size=ROWB,
                    single_packet=False, queue_num=next_q())
                t = wp.tile([P, nh, K], f32, tag="t")
                for h in range(nh):
                    nc.vector.scalar_tensor_tensor(
                        out=t[:, h, :],
                        in0=g[:, :, 256 + h].bitcast(f16),
                        scalar=adst_ap_fn(h),
                        in1=mk[:, t0:t0 + K],
                        op0=Alu.add, op1=Alu.add)
                nc.vector.scalar_tensor_tensor(
                    out=t[:], in0=t[:], scalar=NEG, in1=t[:],
                    op0=Alu.mult, op1=Alu.max)
                # tmp holds, per edge slot k: [w*g (256) | w per head (nh)]
                tmp = wp.tile([P, K, 256 + nh], f16, tag="tmp")
                nc.scalar.activation(
                    tmp[:, :, 256:256 + nh].rearrange("p k h -> p h k"),
                    t[:], Act.Exp)
                nc.vector.tensor_tensor(
                    out=tmp[:, :, 0:256].rearrange("p k (h c) -> p k h c", h=nh),
                    in0=g[:, :, 0:256].bitcast(f16)
                         .rearrange("p k (h c) -> p k h c", h=nh),
                    in1=tmp[:, :, 256:256 + nh][:, :, :, None].to_broadcast(
                        [P, K, nh, 256 // nh]),
                    op=Alu.mult)
                ps = psa.tile([P, 256 + nh], f32, space="PSUM", tag="agg")
                for k in range(K):
                    nc.tensor.matmul(ps[:], id16[:], tmp[:, k, :],
                                     start=(k == 0),
                                     stop=(stop_last and k == K - 1))
                return ps

            def b_phase(tfa, aggB, layer):
                nh = H if layer == 1 else 1
                t0 = 0
                for j in range(NBLK):
                    K = KbG[j]
                    if layer == 1:
                        fn = lambda h, j=j: adstB1[:, j, h:h + 1]
                    else:
                        fn = lambda h, j=j: adstB2[:, j:j + 1]
                    ps = agg_block(K, t0, giB, mkB, tfa[:], nh, fn)
                    stg = sp.tile([P, ROWB], u16, tag="stage")
                    nc.vector.tensor_copy(
                        out=stg[:, 0:256 + nh].bitcast(f16),
                        in_=ps[:])
                    nc.sync.dma_start(aggB[j * P:(j + 1) * P, :], stg[:])
                    t0 += K

            def a_phase(tfb, aggB, layer):
                nh = H if layer == 1 else 1
                t0 = 0
                for i in range(NBLK):
                    K = KaG[i]
                    if layer == 1:
                        fn = lambda h, i=i: adst1[:, i, h:h + 1]
                    else:
                        fn = lambda h, i=i: adst2[:, i:i + 1]
                    ps = agg_block(K, t0, giA, mkA, tfb[:], nh, fn,
                                   stop_last=False)
                    # merge gathered B aggregate (features + denoms) in PSUM
                    c0 = (i // OWN_CHUNK) * OWN_CHUNK
                    if i % OWN_CHUNK == 0:
                        nb = min(OWN_CHUNK, NBLK - c0)
                        gb = op_.tile([P, OWN_CHUNK, ROWB], u16, tag="aggbg")
                        nc.gpsimd.dma_gather(
                            out_ap=gb[:, 0:nb, :], in_ap=aggB[:],
                            idxs_ap=gab[:, c0 * 8:(c0 + nb) * 8],
                            num_idxs=nb * P, num_idxs_reg=nb * P,
                            elem_size=ROWB, single_packet=False,
                            queue_num=next_q())
                        a_phase.gb = gb
                    gb = a_phase.gb
                    jj = i - c0
                    nc.tensor.matmul(
                        ps[:], id16[:],
                        gb[:, jj, 0:256 + nh].bitcast(f16),
                        start=False, stop=True)
                    rec = wp.tile([P, nh], f32, tag="rec")
                    nc.vector.reciprocal(rec[:], ps[:, 256:256 + nh])
                    xv = wp.tile([P, 256], f32, tag="xdiv")
                    nc.vector.tensor_tensor(
                        out=xv[:].rearrange("p (h c) -> p h c", h=nh),
                        in0=ps[:, 0:256].rearrange("p (h c) -> p h c", h=nh),
                        in1=rec[:, :, None].to_broadcast([P, nh, 256 // nh]),
                        op=Alu.mult)
                    if layer == 1:
                        epilogue1(i, xv)
                    else:
                        epilogue2(i, xv)
                    t0 += K

            def epilogue1(i, xv):
                # z = elu(xv); stage [z f16 256 | asrc2 | adst2]
                u = wp.tile([P, 256], f32, tag="eluu")
                nc.vector.tensor_scalar_min(out=u[:], in0=xv[:], scalar1=0.0)
                e = wp.tile([P, 256], f32, tag="elue")
                nc.scalar.activation(e[:], u[:], Act.Exp)
                stg = sp.tile([P, ROWB], u16, tag="stage")
                z16 = stg[:, 0:256].bitcast(f16)
                nc.vector.scalar_tensor_tensor(
                    out=z16, in0=e[:], scalar=-1.0, in1=xv[:],
                    op0=Alu.add, op1=Alu.max)
                # alpha2 = z @ [ws2|wd2] via fused mult+row-accumulate
                pa = wp.tile([P, 2], f32, tag="pa")
                for cch in range(2):
                    scr = wp.tile([P, 256], f32, tag="a2scr")
                    nc.vector.scalar_tensor_tensor(
                        out=scr[:], in0=z16, scalar=1.0,
                        in1=wsdrep[:, cch, :],
                        op0=Alu.mult, op1=Alu.mult,
                        accum_out=pa[:, cch:cch + 1])
                nc.vector.tensor_copy(out=stg[:, 256:258].bitcast(f16),
                                      in_=pa[:])
                nc.vector.tensor_copy(out=adst2[:, i:i + 1], in_=pa[:, 1:2])
                rows = min(NPC - i * P, P)
                if rows > 0:
                    nc.sync.dma_start(tab_own2[i * P:i * P + rows, :],
                                      stg[0:rows, :])
                if i == CBLK - 1:
                    nc.gpsimd.collective_compute(
                        "AllGather", Alu.bypass,
                        replica_groups=[list(range(NC))],
                        ins=[tab_own2[0:CH0, :].opt()],
                        outs=[tfa2.opt()])

            def epilogue2(i, xv):
                po = pss.tile([P, OUT], f32, space="PSUM", tag="out2")
                for cch in range(2):
                    pt = pst.tile([P, P], f32, space="PSUM", tag="tpose")
                    nc.tensor.transpose(pt[:], xv[:, cch * P:(cch + 1) * P],
                                        id32[:])
                    xt = wp.tile([P, P], f32, tag="xt")
                    nc.vector.tensor_copy(out=xt[:], in_=pt[:])
                    nc.tensor.matmul(po[:], xt[:], w2c[:, cch, :],
                                     start=(cch == 0), stop=False)
                nc.tensor.matmul(po[:], ones1[:], b2r[:],
                                 start=False, stop=True)
                # log_softmax over 64 cols
                m = wp.tile([P, 1], f32, tag="lsm")
                nc.vector.reduce_max(m[:], po[:], axis=mybir.AxisListType.X)
                sft = wp.tile([P, OUT], f32, tag="lss")
                nc.vector.tensor_scalar_sub(out=sft[:], in0=po[:], scalar1=m[:])
                ex = wp.tile([P, OUT], f32, tag="lse")
                sm = wp.tile([P, 1], f32, tag="lsum")
                nc.scalar.activation(ex[:], sft[:], Act.Exp, accum_out=sm[:])
                ls = wp.tile([P, 1], f32, tag="lls")
                nc.scalar.activation(ls[:], sm[:], Act.Ln)
                res = wp.tile([P, OUT], f32, tag="lres")
                nc.vector.tensor_scalar_sub(out=res[:], in0=sft[:], scalar1=ls[:])
                nc.sync.dma_start(t_out.ap()[i * P:(i + 1) * P, :], res[:])

            # ---- layer 1 ----
            load_adstB(tab_own1, 1)
            b_phase(tfa1, aggB1, 1)
            a_phase(tfb1, aggB1, 1)

            # ---- finish layer-2 table AllGather (chunk1) ----
            nc.gpsimd.collective_compute(
                "AllGather", Alu.bypass,
                replica_groups=[list(range(NC))],
                ins=[tab_own2[CH0:NPC, :].opt()],
                outs=[tfb2.opt()])

            # ---- layer 2 ----
            load_adstB(tab_own2, 2)
            b_phase(tfa2, aggB2, 2)
            a_phase(tfb2, aggB2, 2)

    nc.compile()
    return nc


# --------------------------------------------------------------------------
# entry point
# --------------------------------------------------------------------------

def kernel(**inputs):
    adj = np.asarray(inputs["adj"]).astype(np.int64)
    key = adj.tobytes()[:64] + adj.tobytes()[-64:]
    if "plan" not in _CACHE or _CACHE.get("key") != key:
        KaG, KbG, per_core = _preprocess(adj)
        nc = _build_program(KaG, KbG)
        _CACHE.update(plan=(KaG, KbG, per_core), nc=nc, key=key)
    KaG, KbG, per_core = _CACHE["plan"]
    nc = _CACHE["nc"]

    maps = _host_tensors(inputs, per_core)
    res = bass_utils.run_bass_kernel_spmd(nc, maps, core_ids=list(range(NC)))

    out = np.empty((N, OUT), np.float32)
    for c in range(NC):
        o = res.results[c]["out"][:NPC]
        out[c * NPC + per_core[c]["permA"]] = o
    return out
